# revision 9
# baseline (speedup 1.0000x reference)
import sys, os, math
sys.path.insert(0, '/opt/trn_rl_repo')
import numpy as np

N_NODES = 50000
N_CORES = 8
NPAD = 50176            # 8 * 6272
NSH = 6272              # nodes per core
NBLK = 49               # 128-node blocks per core
GCH = 256             # edges per gather chunk (>256 wedges SWDGE)

_cache = {}

def _compose_weff(p):
    """Collapse 4 weight-normed linears + out proj into one linear (fp64)."""
    W = None
    b = None
    for v, g, bb in zip(p['v'], p['g'], p['b']):
        v = np.asarray(v, np.float64); g = np.asarray(g, np.float64)
        bb = np.asarray(bb, np.float64)
        Wl = g[:, None] * v / np.linalg.norm(v, axis=1, keepdims=True)
        if W is None:
            W, b = Wl, bb.copy()
        else:
            W = Wl @ W
            b = Wl @ b + bb
    Wo = np.asarray(p['W_out'], np.float64); bo = np.asarray(p['b_out'], np.float64)
    W = Wo @ W
    b = Wo @ b + bo
    return W, b   # [out_c, din], [out_c]


def _build_edge_structure(edge_index):
    src = np.asarray(edge_index[0], np.int64)
    dst = np.asarray(edge_index[1], np.int64)
    E = src.shape[0]
    core = dst // NSH
    blk = (dst - core * NSH) // 128
    dstl = (dst - core * NSH - blk * 128)
    # per (core, block) edge lists
    order = np.lexsort((blk, core))
    so_src, so_dstl, so_core, so_blk = src[order], dstl[order], core[order], blk[order]
    # counts per core/block
    cnts = np.zeros((N_CORES, NBLK), np.int64)
    np.add.at(cnts, (so_core, so_blk), 1)
    tiles_b = np.maximum(1, (cnts.max(axis=0) + 127) // 128)  # [NBLK] shared tile counts
    n_chunks = int(tiles_b.sum())
    EPAD = n_chunks * 128
    # allocate per-core padded arrays
    idxs = np.zeros((N_CORES, EPAD), np.int64)
    pars = np.zeros((N_CORES, EPAD), np.float32)
    dls  = np.full((N_CORES, EPAD), -1.0, np.float32)
    pos  = np.zeros((N_CORES, EPAD), np.int64)   # original edge id (unused)
    attr_i = np.full((N_CORES, EPAD), -1, np.int64)  # index into original edge arrays
    blk_start = np.concatenate([[0], np.cumsum(tiles_b)]) * 128  # edge offset per block
    # fill
    ptr = np.zeros((N_CORES, NBLK), np.int64)
    # compute start offsets of each (core, block) group in sorted arrays
    grp_off = np.zeros((N_CORES, NBLK + 1), np.int64)
    for k in range(N_CORES):
        sel = so_core == k
        bcnt = np.bincount(so_blk[sel], minlength=NBLK)
        grp_off[k, 1:] = np.cumsum(bcnt)
    base_k = np.searchsorted(so_core, np.arange(N_CORES))
    for k in range(N_CORES):
        for b in range(NBLK):
            s, e = grp_off[k, b] + base_k[k], grp_off[k, b + 1] + base_k[k]
            n = e - s
            o = blk_start[b]
            idxs[k, o:o+n] = so_src[s:e] >> 1
            pars[k, o:o+n] = (so_src[s:e] & 1).astype(np.float32)
            dls[k, o:o+n] = so_dstl[s:e].astype(np.float32)
            attr_i[k, o:o+n] = order[s:e]
    # counts per node (for mean)
    cnt_node = np.bincount(dst, minlength=NPAD).astype(np.float32)
    cnt_recip = 1.0 / np.maximum(cnt_node, 1.0)
    # block -> chunk ranges
    blk_chunk0 = (blk_start // 128)
    return dict(EPAD=EPAD, n_chunks=n_chunks, tiles_b=tiles_b, blk_chunk0=blk_chunk0,
                idxs=idxs, pars=pars, dls=dls, attr_i=attr_i, cnt_recip=cnt_recip)


def _wrap_idx16(idx_lin):
    """[E] int -> [128, E/16] int16 wrapped in 16 partitions, replicated x8."""
    E = idx_lin.shape[0]
    w = idx_lin.reshape(E // 16, 16).T.astype(np.int16)  # [16, E/16]
    return np.tile(w, (8, 1)).copy()


def _wrap_dstl(d):
    E = d.shape[0]
    return np.ascontiguousarray(d.reshape(E // 128, 128).T)  # [128, n_chunks]


def _build_program(layer, ES, dt, bass, bacc, tile, mybir):
    """Build one layer's SPMD program."""
    EPAD, n_chunks, tiles_b, blk_chunk0 = ES['EPAD'], ES['n_chunks'], ES['tiles_b'], ES['blk_chunk0']
    DIN = 64 if layer == 0 else 128     # x feature width
    DOUT = 128 if layer == 0 else 64
    KX = 68 if layer == 0 else 128      # lhsT rows for main matmul (L1: 64x + sin/cos/dist/ones)
    n_tiles = (EPAD + 511) // 512
    nc = bacc.Bacc("TRN2", target_bir_lowering=False, debug=False, num_devices=N_CORES)
    f32, bf16, i16 = mybir.dt.float32, mybir.dt.bfloat16, mybir.dt.int16
    xtab = nc.dram_tensor("xtab", [NPAD // 2, 256], bf16, kind="ExternalInput")
    idx = nc.dram_tensor("idx", [128, EPAD // 16], i16, kind="ExternalInput")
    par = nc.dram_tensor("par", [1, EPAD], f32, kind="ExternalInput")
    dstl = nc.dram_tensor("dstl", [128, n_chunks], f32, kind="ExternalInput")
    efeat = nc.dram_tensor("efeat", [4, EPAD], bf16, kind="ExternalInput")
    iota = nc.dram_tensor("iota", [128, 128], f32, kind="ExternalInput")
    ident = nc.dram_tensor("ident", [128, 128], bf16, kind="ExternalInput")
    wmain = nc.dram_tensor("wmain", [KX, DOUT], bf16, kind="ExternalInput")
    we2 = nc.dram_tensor("we2", [4, DOUT], bf16, kind="ExternalInput")  # L2 only (dummy in L1)
    wagg = nc.dram_tensor("wagg", [128 if layer == 0 else 65, DOUT], bf16, kind="ExternalInput")
    bcol = nc.dram_tensor("bcol", [128, 1], f32, kind="ExternalInput")  # L1 only
    cntr = nc.dram_tensor("cntr", [1, NSH], f32, kind="ExternalInput")
    if layer == 0:
        outt = nc.dram_tensor("out", [NSH, 128], bf16, kind="ExternalOutput")
    else:
        outt = nc.dram_tensor("out", [NSH, 64], f32, kind="ExternalOutput")

    with tile.TileContext(nc) as tc:
        with tc.tile_pool(name="persist", bufs=1) as pp:
            idx_sb = pp.tile([128, EPAD // 16], i16); nc.sync.dma_start(idx_sb[:], idx.ap())
            dstl_sb = pp.tile([128, n_chunks], f32); nc.sync.dma_start(dstl_sb[:], dstl.ap())
            iota_sb = pp.tile([128, 128], f32); nc.sync.dma_start(iota_sb[:], iota.ap())
            ident_sb = pp.tile([128, 128], bf16); nc.sync.dma_start(ident_sb[:], ident.ap())
            wmain_sb = pp.tile([KX, DOUT], bf16); nc.sync.dma_start(wmain_sb[:], wmain.ap())
            we2_sb = pp.tile([4, DOUT], bf16); nc.sync.dma_start(we2_sb[:], we2.ap())
            wagg_sb = pp.tile([wagg.shape[0], DOUT], bf16); nc.sync.dma_start(wagg_sb[:], wagg.ap())
            bcol_sb = pp.tile([128, 1], f32); nc.sync.dma_start(bcol_sb[:], bcol.ap())
            cntr_sb = pp.tile([1, NSH], f32); nc.sync.dma_start(cntr_sb[:], cntr.ap())
            cntB = pp.tile([128, NSH], f32)
            nc.gpsimd.partition_broadcast(cntB[:], cntr_sb[:])
            accT = pp.tile([DOUT, NSH], f32)
            nc.vector.memset(accT[:], 0.0)

            # chunk -> block maps
            chunk_blk = np.zeros(n_chunks, np.int64)
            for b in range(NBLK):
                chunk_blk[blk_chunk0[b]: blk_chunk0[b] + tiles_b[b]] = b

            with tc.tile_pool(name="gat", bufs=2) as gp, \
                 tc.tile_pool(name="sm", bufs=3) as sp, \
                 tc.tile_pool(name="ph", bufs=3, space="PSUM") as php, \
                 tc.tile_pool(name="pacc", bufs=2, space="PSUM") as pap:
                acc_psum = {}
                n_g = (EPAD + GCH - 1) // GCH
                for g in range(n_g):
                    e0 = g * GCH
                    ecnt = min(GCH, EPAD - e0)
                    gt = gp.tile([128, 2, ecnt], bf16, tag="g", name=f"g{g}")
                    nc.gpsimd.dma_gather(
                        gt[:], xtab.ap(), idx_sb[:, e0 // 16:(e0 + ecnt) // 16],
                        num_idxs=ecnt, num_idxs_reg=ecnt, elem_size=256, transpose=True)
                    for t0 in range(0, ecnt, 512):
                        te = min(512, ecnt - t0)
                        et = e0 + t0
                        # parity mask
                        pr = sp.tile([1, 512], f32, tag="pr")
                        nc.sync.dma_start(pr[:, :te], par.ap()[:, et:et + te])
                        mk = sp.tile([128, 512], f32, tag="mk")
                        nc.gpsimd.partition_broadcast(mk[:DIN, :te], pr[:, :te])
                        g0 = gt[:, 0, t0:t0 + te]
                        g1 = gt[:, 1, t0:t0 + te]
                        dd = sp.tile([128, 512], f32, tag="dd")
                        nc.vector.tensor_tensor(dd[:DIN, :te], g1[:DIN], g0[:DIN], mybir.AluOpType.subtract)
                        nc.vector.tensor_tensor(dd[:DIN, :te], dd[:DIN, :te], mk[:DIN, :te], mybir.AluOpType.mult)
                        nc.vector.tensor_tensor(g0[:DIN], g0[:DIN], dd[:DIN, :te], mybir.AluOpType.add)
                        if layer == 0:
                            nc.sync.dma_start(gt[64:68, 0, t0:t0 + te], efeat.ap()[:, et:et + te])
                            ef = None
                        else:
                            ef = sp.tile([4, 512], bf16, tag="ef")
                            nc.sync.dma_start(ef[:, :te], efeat.ap()[:, et:et + te])
                        ph = php.tile([128, 512], f32, tag="ph")
                        nsub = te // 128
                        for c in range(nsub):
                            ck = (et + c * 128) // 128
                            po = ph[:, c * DOUT:(c + 1) * DOUT]
                            if layer == 0:
                                nc.tensor.matmul(po, gt[:KX, 0, t0 + c * 128: t0 + (c + 1) * 128],
                                                 wmain_sb[:], start=True, stop=True)
                            else:
                                nc.tensor.matmul(po, gt[:KX, 0, t0 + c * 128: t0 + (c + 1) * 128],
                                                 wmain_sb[:], start=True, stop=False)
                                nc.tensor.matmul(po, ef[:, c * 128:(c + 1) * 128],
                                                 we2_sb[:], start=False, stop=True)
                        hs = sp.tile([128, 512], bf16, tag="hs")
                        fn = mybir.ActivationFunctionType.Gelu if layer == 0 else mybir.ActivationFunctionType.Copy
                        nc.scalar.activation(hs[:, :te], ph[:, :te], fn)
                        for c in range(nsub):
                            ck = (et + c * 128) // 128
                            b = int(chunk_blk[ck])
                            S = sp.tile([128, 128], bf16, tag="S")
                            nc.vector.tensor_scalar(S[:], iota_sb[:], dstl_sb[:, ck:ck + 1], None,
                                                    mybir.AluOpType.is_equal)
                            first = ck == blk_chunk0[b]
                            last = ck == blk_chunk0[b] + tiles_b[b] - 1
                            if first:
                                acc_psum[b] = pap.tile([DOUT, 128], f32, tag="acc", name=f"accp{b}")
                            nc.tensor.matmul(acc_psum[b][:], hs[:, c * DOUT:(c + 1) * DOUT], S[:],
                                             start=first, stop=last)
                            if last:
                                nc.vector.tensor_copy(accT[:, b * 128:(b + 1) * 128], acc_psum[b][:])
                                del acc_psum[b]

            # mean + node update
            with tc.tile_pool(name="upd", bufs=2) as up, \
                 tc.tile_pool(name="pupd", bufs=2, space="PSUM") as pup:
                if layer == 0:
                    meanT = pp.tile([128, NSH], bf16)
                    nc.vector.tensor_tensor(meanT[:], accT[:], cntB[:], mybir.AluOpType.mult)
                    x2T = pp.tile([128, NSH], bf16)
                    for o in range(0, NSH, 512):
                        oe = min(512, NSH - o)
                        pu = pup.tile([128, 512], f32, tag="pu")
                        nc.tensor.matmul(pu[:, :oe], wagg_sb[:], meanT[:, o:o + oe], start=True, stop=True)
                        nc.scalar.activation(x2T[:, o:o + oe], pu[:, :oe],
                                             mybir.ActivationFunctionType.Tanh, bias=bcol_sb[:])
                    # write x2T blocks (feature-major); host transposes during relay
                    for b in range(NBLK):
                        xo = up.tile([128, 128], bf16, tag="xo")
                        nc.vector.tensor_copy(xo[:], x2T[:, b * 128:(b + 1) * 128])
                        nc.sync.dma_start(outt.ap()[b * 128:(b + 1) * 128, :], xo[:])
                else:
                    meanT = pp.tile([65, NSH], bf16)
                    nc.vector.memset(meanT[64:65, :], 1.0)
                    nc.vector.tensor_tensor(meanT[:64, :], accT[:64, :], cntB[:64, :], mybir.AluOpType.mult)
                    for b in range(NBLK):
                        pu = pup.tile([128, 64], f32, tag="pu")
                        nc.tensor.matmul(pu[:], meanT[:, b * 128:(b + 1) * 128], wagg_sb[:],
                                         start=True, stop=True)
                        xo = up.tile([128, 64], f32, tag="xo")
                        nc.vector.tensor_copy(xo[:], pu[:])
                        nc.sync.dma_start(outt.ap()[b * 128:(b + 1) * 128, :], xo[:])
    nc.compile()
    return nc


def _prep_host(x, edge_attr, edge_distance, params, edge_index):
    import concourse.bass as bass, concourse.bacc as bacc
    import concourse.tile as tile, concourse.mybir as mybir
    ES = _build_edge_structure(edge_index)
    EPAD = ES['EPAD']
    W1, b1 = _compose_weff(params[0])   # [128, 67], [128]
    W2, b2 = _compose_weff(params[1])   # [64, 131], [64]

    import ml_dtypes
    def tobf(a):
        return np.asarray(a, np.float32).astype(ml_dtypes.bfloat16)

    # L1 weights: rows 0-63 W1x^T, 64 sin, 65 cos, 66 dist, 67 bias
    w1 = np.zeros((68, 128), np.float32)
    w1[:64] = W1[:, :64].T
    w1[64] = W1[:, 64]; w1[65] = W1[:, 65]; w1[66] = W1[:, 66]; w1[67] = b1
    w2m = np.ascontiguousarray(W2[:, :128].T, dtype=np.float32)  # [128, 64]
    we2 = np.zeros((4, 64), np.float32)
    we2[0] = W2[:, 128]; we2[1] = W2[:, 129]; we2[2] = W2[:, 130]; we2[3] = b2
    p1, p2 = params[0], params[1]
    wagg1T = np.asarray(p1['W_agg'], np.float32).T.copy()          # [128,128] lhsT
    b1col = (np.asarray(p1['b_agg'], np.float32) + np.asarray(p1['bias'], np.float32)).reshape(128, 1)
    wagg2 = np.zeros((65, 64), np.float32)
    wagg2[:64] = np.asarray(p2['W_agg'], np.float32).T
    wagg2[64] = np.asarray(p2['b_agg'], np.float32) + np.asarray(p2['bias'], np.float32)

    # x table L1: pair rows [x_2j|0|x_2j+1|0]
    xp = np.zeros((NPAD, 128), np.float32)
    xp[:N_NODES, :64] = np.asarray(x, np.float32)
    xtab1 = tobf(xp.reshape(NPAD // 2, 256))

    ea = np.asarray(edge_attr, np.float32); ed = np.asarray(edge_distance, np.float32)
    ang = ea * (math.pi / 180.0)
    sin_a, cos_a = np.sin(ang), np.cos(ang)
    iota = np.broadcast_to(np.arange(128, dtype=np.float32)[None, :], (128, 128)).copy()
    ident = tobf(np.eye(128, dtype=np.float32))

    per_core = []
    for k in range(N_CORES):
        ai = ES['attr_i'][k]
        val = ai >= 0
        aid = np.where(val, ai, 0)
        ef = np.zeros((4, EPAD), np.float32)
        ef[0] = np.where(val, sin_a[aid], 0.0)
        ef[1] = np.where(val, cos_a[aid], 0.0)
        ef[2] = np.where(val, ed[aid], 0.0)
        ef[3] = 1.0
        per_core.append(dict(
            idx=_wrap_idx16(ES['idxs'][k]),
            par=ES['pars'][k].reshape(1, EPAD),
            dstl=_wrap_dstl(ES['dls'][k]),
            efeat=tobf(ef),
            cntr=ES['cnt_recip'][k * NSH:(k + 1) * NSH].reshape(1, NSH).copy(),
        ))
    consts = dict(iota=iota, ident=ident,
                  w1=tobf(w1), w2m=tobf(w2m), we2=tobf(we2),
                  wagg1T=tobf(wagg1T), b1col=b1col, wagg2=tobf(wagg2))
    return ES, xtab1, per_core, consts


def kernel(x, edge_attr, edge_distance, params, edge_index):
    x = np.asarray(x); edge_attr = np.asarray(edge_attr)
    edge_distance = np.asarray(edge_distance); edge_index = np.asarray(edge_index)
    import concourse.bass as bass, concourse.bacc as bacc
    import concourse.tile as tile, concourse.mybir as mybir
    from concourse.bass_utils import run_bass_kernel_spmd
    ES, xtab1, per_core, C = _prep_host(x, edge_attr, edge_distance, params, edge_index)
    key = ('prog', ES['EPAD'])
    if key not in _cache:
        _cache[key] = (_build_program(0, ES, None, bass, bacc, tile, mybir),
                       _build_program(1, ES, None, bass, bacc, tile, mybir))
    nc1, nc2 = _cache[key]
    dummy_bcol = np.zeros((128, 1), np.float32)
    dummy_we2 = np.zeros((4, 128), np.float32)

    import ml_dtypes
    def tobf(a):
        return np.asarray(a, np.float32).astype(ml_dtypes.bfloat16)

    in1 = [dict(xtab=xtab1, idx=pc['idx'], par=pc['par'], dstl=pc['dstl'],
                efeat=pc['efeat'], iota=C['iota'], ident=C['ident'],
                wmain=C['w1'], we2=tobf(dummy_we2), wagg=C['wagg1T'],
                bcol=C['b1col'], cntr=pc['cntr']) for pc in per_core]
    import time as _time
    _tr = bool(os.environ.get('BASS_GNN_TRACE'))
    _t0 = _time.time()
    r1 = run_bass_kernel_spmd(nc1, in1, core_ids=list(range(N_CORES)), trace=_tr)
    _t1 = _time.time()
    # per-core out rows b*128..(b+1)*128 hold x2T block b (feature-major) -> transpose to node-major
    x2 = np.concatenate([
        np.ascontiguousarray(np.transpose(
            r1.results[k]['out'].reshape(NBLK, 128, 128), (0, 2, 1))).reshape(NSH, 128)
        for k in range(N_CORES)], axis=0)  # [NPAD,128] bf16 node-major
    xtab2 = np.ascontiguousarray(x2.reshape(NPAD // 2, 256))
    in2 = [dict(xtab=xtab2, idx=pc['idx'], par=pc['par'], dstl=pc['dstl'],
                efeat=pc['efeat'], iota=C['iota'], ident=C['ident'],
                wmain=C['w2m'], we2=C['we2'], wagg=C['wagg2'],
                bcol=dummy_bcol, cntr=pc['cntr']) for pc in per_core]
    _t2 = _time.time()
    r2 = run_bass_kernel_spmd(nc2, in2, core_ids=list(range(N_CORES)), trace=_tr)
    _t3 = _time.time()
    out = np.concatenate([r2.results[k]['out'] for k in range(N_CORES)], axis=0)
    e1 = getattr(r1, 'exec_time_ns', None) or int((_t1 - _t0) * 1e9)
    e2 = getattr(r2, 'exec_time_ns', None) or int((_t3 - _t2) * 1e9)
    kernel._last_exec_ns = [e1, e2]
    return np.ascontiguousarray(out[:N_NODES].astype(np.float32))


# revision 10
# speedup vs baseline: 1.0912x; 1.0912x over previous
import sys, os, math
sys.path.insert(0, '/opt/trn_rl_repo')
import numpy as np

N_NODES = 50000
N_CORES = 8
NPAD = 50176            # 8 * 6272
NSH = 6272              # nodes per core
NBLK = 49               # 128-node blocks per core
GCH = 512             # edges per gather chunk (1024 wedges SWDGE; 512 under test)

_cache = {}

def _compose_weff(p):
    """Collapse 4 weight-normed linears + out proj into one linear (fp64)."""
    W = None
    b = None
    for v, g, bb in zip(p['v'], p['g'], p['b']):
        v = np.asarray(v, np.float64); g = np.asarray(g, np.float64)
        bb = np.asarray(bb, np.float64)
        Wl = g[:, None] * v / np.linalg.norm(v, axis=1, keepdims=True)
        if W is None:
            W, b = Wl, bb.copy()
        else:
            W = Wl @ W
            b = Wl @ b + bb
    Wo = np.asarray(p['W_out'], np.float64); bo = np.asarray(p['b_out'], np.float64)
    W = Wo @ W
    b = Wo @ b + bo
    return W, b   # [out_c, din], [out_c]


def _build_edge_structure(edge_index):
    src = np.asarray(edge_index[0], np.int64)
    dst = np.asarray(edge_index[1], np.int64)
    E = src.shape[0]
    core = dst // NSH
    blk = (dst - core * NSH) // 128
    dstl = (dst - core * NSH - blk * 128)
    # per (core, block) edge lists
    order = np.lexsort((blk, core))
    so_src, so_dstl, so_core, so_blk = src[order], dstl[order], core[order], blk[order]
    # counts per core/block
    cnts = np.zeros((N_CORES, NBLK), np.int64)
    np.add.at(cnts, (so_core, so_blk), 1)
    tiles_b = np.maximum(1, (cnts.max(axis=0) + 127) // 128)  # [NBLK] shared tile counts
    n_chunks = int(tiles_b.sum())
    EPAD = n_chunks * 128
    # allocate per-core padded arrays
    idxs = np.zeros((N_CORES, EPAD), np.int64)
    pars = np.zeros((N_CORES, EPAD), np.float32)
    dls  = np.full((N_CORES, EPAD), -1.0, np.float32)
    pos  = np.zeros((N_CORES, EPAD), np.int64)   # original edge id (unused)
    attr_i = np.full((N_CORES, EPAD), -1, np.int64)  # index into original edge arrays
    blk_start = np.concatenate([[0], np.cumsum(tiles_b)]) * 128  # edge offset per block
    # fill
    ptr = np.zeros((N_CORES, NBLK), np.int64)
    # compute start offsets of each (core, block) group in sorted arrays
    grp_off = np.zeros((N_CORES, NBLK + 1), np.int64)
    for k in range(N_CORES):
        sel = so_core == k
        bcnt = np.bincount(so_blk[sel], minlength=NBLK)
        grp_off[k, 1:] = np.cumsum(bcnt)
    base_k = np.searchsorted(so_core, np.arange(N_CORES))
    for k in range(N_CORES):
        for b in range(NBLK):
            s, e = grp_off[k, b] + base_k[k], grp_off[k, b + 1] + base_k[k]
            n = e - s
            o = blk_start[b]
            idxs[k, o:o+n] = so_src[s:e] >> 1
            pars[k, o:o+n] = (so_src[s:e] & 1).astype(np.float32)
            dls[k, o:o+n] = so_dstl[s:e].astype(np.float32)
            attr_i[k, o:o+n] = order[s:e]
    # counts per node (for mean)
    cnt_node = np.bincount(dst, minlength=NPAD).astype(np.float32)
    cnt_recip = 1.0 / np.maximum(cnt_node, 1.0)
    # block -> chunk ranges
    blk_chunk0 = (blk_start // 128)
    return dict(EPAD=EPAD, n_chunks=n_chunks, tiles_b=tiles_b, blk_chunk0=blk_chunk0,
                idxs=idxs, pars=pars, dls=dls, attr_i=attr_i, cnt_recip=cnt_recip)


def _wrap_idx16(idx_lin):
    """[E] int -> [128, E/16] int16 wrapped in 16 partitions, replicated x8."""
    E = idx_lin.shape[0]
    w = idx_lin.reshape(E // 16, 16).T.astype(np.int16)  # [16, E/16]
    return np.tile(w, (8, 1)).copy()


def _wrap_dstl(d):
    E = d.shape[0]
    return np.ascontiguousarray(d.reshape(E // 128, 128).T)  # [128, n_chunks]


def _build_program(layer, ES, dt, bass, bacc, tile, mybir):
    """Build one layer's SPMD program."""
    EPAD, n_chunks, tiles_b, blk_chunk0 = ES['EPAD'], ES['n_chunks'], ES['tiles_b'], ES['blk_chunk0']
    DIN = 64 if layer == 0 else 128     # x feature width
    DOUT = 128 if layer == 0 else 64
    KX = 68 if layer == 0 else 128      # lhsT rows for main matmul (L1: 64x + sin/cos/dist/ones)
    n_tiles = (EPAD + 511) // 512
    nc = bacc.Bacc("TRN2", target_bir_lowering=False, debug=False, num_devices=N_CORES)
    f32, bf16, i16 = mybir.dt.float32, mybir.dt.bfloat16, mybir.dt.int16
    xtab = nc.dram_tensor("xtab", [NPAD // 2, 256], bf16, kind="ExternalInput")
    idx = nc.dram_tensor("idx", [128, EPAD // 16], i16, kind="ExternalInput")
    par = nc.dram_tensor("par", [1, EPAD], f32, kind="ExternalInput")
    dstl = nc.dram_tensor("dstl", [128, n_chunks], f32, kind="ExternalInput")
    efeat = nc.dram_tensor("efeat", [4, EPAD], bf16, kind="ExternalInput")
    iota = nc.dram_tensor("iota", [128, 128], f32, kind="ExternalInput")
    ident = nc.dram_tensor("ident", [128, 128], bf16, kind="ExternalInput")
    wmain = nc.dram_tensor("wmain", [KX, DOUT], bf16, kind="ExternalInput")
    we2 = nc.dram_tensor("we2", [4, DOUT], bf16, kind="ExternalInput")  # L2 only (dummy in L1)
    wagg = nc.dram_tensor("wagg", [128 if layer == 0 else 65, DOUT], bf16, kind="ExternalInput")
    bcol = nc.dram_tensor("bcol", [128, 1], f32, kind="ExternalInput")  # L1 only
    cntr = nc.dram_tensor("cntr", [1, NSH], f32, kind="ExternalInput")
    if layer == 0:
        outt = nc.dram_tensor("out", [NSH, 128], bf16, kind="ExternalOutput")
    else:
        outt = nc.dram_tensor("out", [NSH, 64], f32, kind="ExternalOutput")

    with tile.TileContext(nc) as tc:
        with tc.tile_pool(name="persist", bufs=1) as pp:
            idx_sb = pp.tile([128, EPAD // 16], i16); nc.sync.dma_start(idx_sb[:], idx.ap())
            dstl_sb = pp.tile([128, n_chunks], f32); nc.sync.dma_start(dstl_sb[:], dstl.ap())
            iota_sb = pp.tile([128, 128], f32); nc.sync.dma_start(iota_sb[:], iota.ap())
            ident_sb = pp.tile([128, 128], bf16); nc.sync.dma_start(ident_sb[:], ident.ap())
            wmain_sb = pp.tile([KX, DOUT], bf16); nc.sync.dma_start(wmain_sb[:], wmain.ap())
            we2_sb = pp.tile([4, DOUT], bf16); nc.sync.dma_start(we2_sb[:], we2.ap())
            wagg_sb = pp.tile([wagg.shape[0], DOUT], bf16); nc.sync.dma_start(wagg_sb[:], wagg.ap())
            bcol_sb = pp.tile([128, 1], f32); nc.sync.dma_start(bcol_sb[:], bcol.ap())
            cntr_sb = pp.tile([1, NSH], f32); nc.sync.dma_start(cntr_sb[:], cntr.ap())
            cntB = pp.tile([128, NSH], f32)
            nc.gpsimd.partition_broadcast(cntB[:], cntr_sb[:])
            accT = pp.tile([DOUT, NSH], f32)
            nc.vector.memset(accT[:], 0.0)

            # chunk -> block maps
            chunk_blk = np.zeros(n_chunks, np.int64)
            for b in range(NBLK):
                chunk_blk[blk_chunk0[b]: blk_chunk0[b] + tiles_b[b]] = b

            with tc.tile_pool(name="gat", bufs=2) as gp, \
                 tc.tile_pool(name="sm", bufs=3) as sp, \
                 tc.tile_pool(name="ph", bufs=3, space="PSUM") as php, \
                 tc.tile_pool(name="pacc", bufs=2, space="PSUM") as pap:
                acc_psum = {}
                n_g = (EPAD + GCH - 1) // GCH
                for g in range(n_g):
                    e0 = g * GCH
                    ecnt = min(GCH, EPAD - e0)
                    gt = gp.tile([128, 2, ecnt], bf16, tag="g", name=f"g{g}")
                    nc.gpsimd.dma_gather(
                        gt[:], xtab.ap(), idx_sb[:, e0 // 16:(e0 + ecnt) // 16],
                        num_idxs=ecnt, num_idxs_reg=ecnt, elem_size=256, transpose=True)
                    for t0 in range(0, ecnt, 512):
                        te = min(512, ecnt - t0)
                        et = e0 + t0
                        # parity mask
                        pr = sp.tile([1, 512], f32, tag="pr")
                        nc.sync.dma_start(pr[:, :te], par.ap()[:, et:et + te])
                        mk = sp.tile([128, 512], f32, tag="mk")
                        nc.gpsimd.partition_broadcast(mk[:DIN, :te], pr[:, :te])
                        g0 = gt[:, 0, t0:t0 + te]
                        g1 = gt[:, 1, t0:t0 + te]
                        dd = sp.tile([128, 512], f32, tag="dd")
                        nc.vector.tensor_tensor(dd[:DIN, :te], g1[:DIN], g0[:DIN], mybir.AluOpType.subtract)
                        nc.vector.tensor_tensor(dd[:DIN, :te], dd[:DIN, :te], mk[:DIN, :te], mybir.AluOpType.mult)
                        nc.vector.tensor_tensor(g0[:DIN], g0[:DIN], dd[:DIN, :te], mybir.AluOpType.add)
                        if layer == 0:
                            nc.sync.dma_start(gt[64:68, 0, t0:t0 + te], efeat.ap()[:, et:et + te])
                            ef = None
                        else:
                            ef = sp.tile([4, 512], bf16, tag="ef")
                            nc.sync.dma_start(ef[:, :te], efeat.ap()[:, et:et + te])
                        ph = php.tile([128, 512], f32, tag="ph")
                        nsub = te // 128
                        for c in range(nsub):
                            ck = (et + c * 128) // 128
                            po = ph[:, c * DOUT:(c + 1) * DOUT]
                            if layer == 0:
                                nc.tensor.matmul(po, gt[:KX, 0, t0 + c * 128: t0 + (c + 1) * 128],
                                                 wmain_sb[:], start=True, stop=True)
                            else:
                                nc.tensor.matmul(po, gt[:KX, 0, t0 + c * 128: t0 + (c + 1) * 128],
                                                 wmain_sb[:], start=True, stop=False)
                                nc.tensor.matmul(po, ef[:, c * 128:(c + 1) * 128],
                                                 we2_sb[:], start=False, stop=True)
                        hs = sp.tile([128, 512], bf16, tag="hs")
                        fn = mybir.ActivationFunctionType.Gelu if layer == 0 else mybir.ActivationFunctionType.Copy
                        nc.scalar.activation(hs[:, :te], ph[:, :te], fn)
                        for c in range(nsub):
                            ck = (et + c * 128) // 128
                            b = int(chunk_blk[ck])
                            S = sp.tile([128, 128], bf16, tag="S")
                            nc.vector.tensor_scalar(S[:], iota_sb[:], dstl_sb[:, ck:ck + 1], None,
                                                    mybir.AluOpType.is_equal)
                            first = ck == blk_chunk0[b]
                            last = ck == blk_chunk0[b] + tiles_b[b] - 1
                            if first:
                                acc_psum[b] = pap.tile([DOUT, 128], f32, tag="acc", name=f"accp{b}")
                            nc.tensor.matmul(acc_psum[b][:], hs[:, c * DOUT:(c + 1) * DOUT], S[:],
                                             start=first, stop=last)
                            if last:
                                nc.vector.tensor_copy(accT[:, b * 128:(b + 1) * 128], acc_psum[b][:])
                                del acc_psum[b]

            # mean + node update
            with tc.tile_pool(name="upd", bufs=2) as up, \
                 tc.tile_pool(name="pupd", bufs=2, space="PSUM") as pup:
                if layer == 0:
                    meanT = pp.tile([128, NSH], bf16)
                    nc.vector.tensor_tensor(meanT[:], accT[:], cntB[:], mybir.AluOpType.mult)
                    x2T = pp.tile([128, NSH], bf16)
                    for o in range(0, NSH, 512):
                        oe = min(512, NSH - o)
                        pu = pup.tile([128, 512], f32, tag="pu")
                        nc.tensor.matmul(pu[:, :oe], wagg_sb[:], meanT[:, o:o + oe], start=True, stop=True)
                        nc.scalar.activation(x2T[:, o:o + oe], pu[:, :oe],
                                             mybir.ActivationFunctionType.Tanh, bias=bcol_sb[:])
                    # write x2T blocks (feature-major); host transposes during relay
                    for b in range(NBLK):
                        xo = up.tile([128, 128], bf16, tag="xo")
                        nc.vector.tensor_copy(xo[:], x2T[:, b * 128:(b + 1) * 128])
                        nc.sync.dma_start(outt.ap()[b * 128:(b + 1) * 128, :], xo[:])
                else:
                    meanT = pp.tile([65, NSH], bf16)
                    nc.vector.memset(meanT[64:65, :], 1.0)
                    nc.vector.tensor_tensor(meanT[:64, :], accT[:64, :], cntB[:64, :], mybir.AluOpType.mult)
                    for b in range(NBLK):
                        pu = pup.tile([128, 64], f32, tag="pu")
                        nc.tensor.matmul(pu[:], meanT[:, b * 128:(b + 1) * 128], wagg_sb[:],
                                         start=True, stop=True)
                        xo = up.tile([128, 64], f32, tag="xo")
                        nc.vector.tensor_copy(xo[:], pu[:])
                        nc.sync.dma_start(outt.ap()[b * 128:(b + 1) * 128, :], xo[:])
    nc.compile()
    return nc


def _prep_host(x, edge_attr, edge_distance, params, edge_index):
    import concourse.bass as bass, concourse.bacc as bacc
    import concourse.tile as tile, concourse.mybir as mybir
    ES = _build_edge_structure(edge_index)
    EPAD = ES['EPAD']
    W1, b1 = _compose_weff(params[0])   # [128, 67], [128]
    W2, b2 = _compose_weff(params[1])   # [64, 131], [64]

    import ml_dtypes
    def tobf(a):
        return np.asarray(a, np.float32).astype(ml_dtypes.bfloat16)

    # L1 weights: rows 0-63 W1x^T, 64 sin, 65 cos, 66 dist, 67 bias
    w1 = np.zeros((68, 128), np.float32)
    w1[:64] = W1[:, :64].T
    w1[64] = W1[:, 64]; w1[65] = W1[:, 65]; w1[66] = W1[:, 66]; w1[67] = b1
    w2m = np.ascontiguousarray(W2[:, :128].T, dtype=np.float32)  # [128, 64]
    we2 = np.zeros((4, 64), np.float32)
    we2[0] = W2[:, 128]; we2[1] = W2[:, 129]; we2[2] = W2[:, 130]; we2[3] = b2
    p1, p2 = params[0], params[1]
    wagg1T = np.asarray(p1['W_agg'], np.float32).T.copy()          # [128,128] lhsT
    b1col = (np.asarray(p1['b_agg'], np.float32) + np.asarray(p1['bias'], np.float32)).reshape(128, 1)
    wagg2 = np.zeros((65, 64), np.float32)
    wagg2[:64] = np.asarray(p2['W_agg'], np.float32).T
    wagg2[64] = np.asarray(p2['b_agg'], np.float32) + np.asarray(p2['bias'], np.float32)

    # x table L1: pair rows [x_2j|0|x_2j+1|0]
    xp = np.zeros((NPAD, 128), np.float32)
    xp[:N_NODES, :64] = np.asarray(x, np.float32)
    xtab1 = tobf(xp.reshape(NPAD // 2, 256))

    ea = np.asarray(edge_attr, np.float32); ed = np.asarray(edge_distance, np.float32)
    ang = ea * (math.pi / 180.0)
    sin_a, cos_a = np.sin(ang), np.cos(ang)
    iota = np.broadcast_to(np.arange(128, dtype=np.float32)[None, :], (128, 128)).copy()
    ident = tobf(np.eye(128, dtype=np.float32))

    per_core = []
    for k in range(N_CORES):
        ai = ES['attr_i'][k]
        val = ai >= 0
        aid = np.where(val, ai, 0)
        ef = np.zeros((4, EPAD), np.float32)
        ef[0] = np.where(val, sin_a[aid], 0.0)
        ef[1] = np.where(val, cos_a[aid], 0.0)
        ef[2] = np.where(val, ed[aid], 0.0)
        ef[3] = 1.0
        per_core.append(dict(
            idx=_wrap_idx16(ES['idxs'][k]),
            par=ES['pars'][k].reshape(1, EPAD),
            dstl=_wrap_dstl(ES['dls'][k]),
            efeat=tobf(ef),
            cntr=ES['cnt_recip'][k * NSH:(k + 1) * NSH].reshape(1, NSH).copy(),
        ))
    consts = dict(iota=iota, ident=ident,
                  w1=tobf(w1), w2m=tobf(w2m), we2=tobf(we2),
                  wagg1T=tobf(wagg1T), b1col=b1col, wagg2=tobf(wagg2))
    return ES, xtab1, per_core, consts


def kernel(x, edge_attr, edge_distance, params, edge_index):
    x = np.asarray(x); edge_attr = np.asarray(edge_attr)
    edge_distance = np.asarray(edge_distance); edge_index = np.asarray(edge_index)
    import concourse.bass as bass, concourse.bacc as bacc
    import concourse.tile as tile, concourse.mybir as mybir
    from concourse.bass_utils import run_bass_kernel_spmd
    ES, xtab1, per_core, C = _prep_host(x, edge_attr, edge_distance, params, edge_index)
    key = ('prog', ES['EPAD'])
    if key not in _cache:
        _cache[key] = (_build_program(0, ES, None, bass, bacc, tile, mybir),
                       _build_program(1, ES, None, bass, bacc, tile, mybir))
    nc1, nc2 = _cache[key]
    dummy_bcol = np.zeros((128, 1), np.float32)
    dummy_we2 = np.zeros((4, 128), np.float32)

    import ml_dtypes
    def tobf(a):
        return np.asarray(a, np.float32).astype(ml_dtypes.bfloat16)

    in1 = [dict(xtab=xtab1, idx=pc['idx'], par=pc['par'], dstl=pc['dstl'],
                efeat=pc['efeat'], iota=C['iota'], ident=C['ident'],
                wmain=C['w1'], we2=tobf(dummy_we2), wagg=C['wagg1T'],
                bcol=C['b1col'], cntr=pc['cntr']) for pc in per_core]
    import time as _time
    _tr = bool(os.environ.get('BASS_GNN_TRACE'))
    _t0 = _time.time()
    r1 = run_bass_kernel_spmd(nc1, in1, core_ids=list(range(N_CORES)), trace=_tr)
    _t1 = _time.time()
    # per-core out rows b*128..(b+1)*128 hold x2T block b (feature-major) -> transpose to node-major
    x2 = np.concatenate([
        np.ascontiguousarray(np.transpose(
            r1.results[k]['out'].reshape(NBLK, 128, 128), (0, 2, 1))).reshape(NSH, 128)
        for k in range(N_CORES)], axis=0)  # [NPAD,128] bf16 node-major
    xtab2 = np.ascontiguousarray(x2.reshape(NPAD // 2, 256))
    in2 = [dict(xtab=xtab2, idx=pc['idx'], par=pc['par'], dstl=pc['dstl'],
                efeat=pc['efeat'], iota=C['iota'], ident=C['ident'],
                wmain=C['w2m'], we2=C['we2'], wagg=C['wagg2'],
                bcol=dummy_bcol, cntr=pc['cntr']) for pc in per_core]
    _t2 = _time.time()
    r2 = run_bass_kernel_spmd(nc2, in2, core_ids=list(range(N_CORES)), trace=_tr)
    _t3 = _time.time()
    out = np.concatenate([r2.results[k]['out'] for k in range(N_CORES)], axis=0)
    e1 = getattr(r1, 'exec_time_ns', None) or int((_t1 - _t0) * 1e9)
    e2 = getattr(r2, 'exec_time_ns', None) or int((_t3 - _t2) * 1e9)
    kernel._last_exec_ns = [e1, e2]
    return np.ascontiguousarray(out[:N_NODES].astype(np.float32))
